# revision 1
# baseline (speedup 1.0000x reference)
"""DEMA (double exponential moving average) Trainium2 Bass kernel.

Problem: x [32, 4096, 512] f32; y = 2*EMA(x) - EMA(EMA(x)) along time axis
(L=4096), alpha=0.1, with y_0 = x_0 initial condition.

Strategy
--------
Data-parallel over batch: 8 cores x 4 batch rows each (no communication).

Per core, the time axis is processed in blocks of T=126 steps. DEMA is a
linear recurrence with a 2-dim state (the two EMA carries c1, c2); one
constant augmented matrix A [128, 128] maps [x_block(126); c1; c2] ->
[dema_block(126); c1'; c2'], so each block is exactly ONE fp32 matmul on the
tensor engine. Blocks chain via the carry rows; the 4 batch rows are 4
independent chains interleaved to keep the PE busy.

Partition layout (compute-engine APs must start at partition 0/32/64/96):
the carries live at partitions 96..97; time rows i map to partition i for
i<96 and i+2 for i>=96. The carry hand-off between consecutive blocks is then
a single [2, 512] copy starting at partition 96 (allowed), and the block
output copy is one full [128, 512] PSUM->SBUF copy. DMAs (no partition
restriction) move the two time-row spans separately.

Expected bottleneck: HBM traffic (64 MB/core) -> memory roofline.
"""

import numpy as np

ALPHA = 0.1
BETA = 1.0 - ALPHA
B_FULL, L, C = 32, 4096, 512
N_CORES = 8
B_PER_CORE = B_FULL // N_CORES  # 4
T = 126  # time steps per block (plus 2 carry rows = 128 partitions)
NFULL = L // T  # 32 full blocks
TAIL = L - NFULL * T  # 64
GRP = 4  # blocks per DMA group (~1 MB transfers)
NGRP = NFULL // GRP  # 8 full groups per batch row
SPLIT = 96  # time rows 0..95 at partitions 0..95; 96..125 at 98..127


def _build_A(dtype=np.float64):
    """Permuted augmented operator (lhsT is its transpose)."""
    i = np.arange(T)
    M = np.zeros((T, T), dtype)
    for r in range(T):
        M[r, : r + 1] = ALPHA * BETA ** (r - np.arange(r + 1))
    d = BETA ** (i + 1.0)
    M2 = M @ M
    Md = M @ d
    A = np.zeros((T + 2, T + 2), dtype)
    A[:T, :T] = 2 * M - M2
    A[:T, T] = 2 * d - Md
    A[:T, T + 1] = -d
    A[T, :T] = M[T - 1, :]
    A[T, T] = BETA**T
    A[T + 1, :T] = M2[T - 1, :]
    A[T + 1, T] = Md[T - 1]
    A[T + 1, T + 1] = BETA**T
    # permute: partition p <- time row p (p<96), carries at 96..97,
    # time rows 96..125 at partitions 98..127
    order = list(range(SPLIT)) + [T, T + 1] + list(range(SPLIT, T))
    return A[np.ix_(order, order)]


def _build_A0():
    """First-block variant: folds the c1 = c2 = x_0 initial condition into
    column 0, so no carry rows need to be DMA'd for block 0 (the carry
    partitions only need to hold finite values)."""
    A = _build_A()
    A0 = A.copy()
    A0[:, 0] += A[:, SPLIT] + A[:, SPLIT + 1]
    A0[:, SPLIT] = 0.0
    A0[:, SPLIT + 1] = 0.0
    return A0


def build_bass(n_batch=B_PER_CORE, ngrp=None, with_tail=True, l_mult=1):
    """Emit the per-core Bass/Tile program. Returns the Bass module.

    l_mult > 1 builds a work-scaled timing variant (longer time axis, no
    tail) with identical per-block structure; only used by test.py."""
    import concourse.bass as bass
    import concourse.bacc as bacc
    import concourse.mybir as mybir
    from concourse import tile

    l_total = L * l_mult
    if ngrp is None:
        ngrp = NGRP if l_mult == 1 else l_total // T // GRP
    if l_mult > 1:
        with_tail = False

    fp32 = mybir.dt.float32
    nc = bacc.Bacc(
        "TRN2", target_bir_lowering=False, debug=False, num_devices=N_CORES
    )

    x = nc.dram_tensor("x", [B_PER_CORE, l_total, C], fp32, kind="ExternalInput")
    # amat[:, 0:128] = steady-state lhsT; amat[:, 128:256] = first-block lhsT
    amat = nc.dram_tensor("amat", [128, 256], fp32, kind="ExternalInput")
    y = nc.dram_tensor("y", [B_PER_CORE, l_total, C], fp32, kind="ExternalOutput")
    x_ap, y_ap = x.ap(), y.ap()

    with tile.TileContext(nc) as tc:
        with (
            tc.tile_pool(name="w", bufs=1) as w_pool,
            tc.tile_pool(name="rhs", bufs=12) as rhs_pool,
            tc.tile_pool(name="out", bufs=8) as out_pool,
            tc.tile_pool(name="psum", bufs=8, space="PSUM") as psum_pool,
        ):
            w = w_pool.tile([128, 256], fp32)
            nc.sync.dma_start(w[:, :], amat.ap()[:, :])

            def load_group(b, g):
                """Allocate rhs tile for (batch b, group g) and DMA x into it."""
                t0 = g * GRP * T
                rt = rhs_pool.tile([128, GRP * C], fp32)
                if g < ngrp:
                    src = x_ap[b, t0 : t0 + GRP * T, :].rearrange(
                        "(blk t) c -> t blk c", t=T
                    )
                    dst = rt[:, :].rearrange("t (blk c) -> t blk c", blk=GRP)
                    nc.sync.dma_start(dst[0:SPLIT], src[0:SPLIT])
                    nc.sync.dma_start(dst[SPLIT + 2 : T + 2], src[SPLIT:T])
                else:
                    # tail group: 64 data rows at partitions 0..63, zero-pad rest
                    nc.sync.dma_start(rt[0:TAIL, 0:C], x_ap[b, t0 : t0 + TAIL, :])
                    nc.gpsimd.memset(rt[TAIL:128, 0:C], 0.0)
                if g == 0:
                    # block 0 uses the A0 matrix (zero carry columns); the
                    # carry partitions just need to be finite
                    nc.gpsimd.memset(rt[SPLIT : SPLIT + 2, 0:C], 0.0)
                return rt

            rhs_cur = [load_group(b, 0) for b in range(n_batch)]

            blk_idx = 0
            n_steps = ngrp + 1 if with_tail else ngrp
            for g in range(n_steps):
                rhs_nxt = [None] * n_batch
                if g < ngrp:
                    for b in range(n_batch):
                        rhs_nxt[b] = load_group(b, g + 1)
                for b in range(n_batch):
                    rt = rhs_cur[b]
                    nblk = GRP if g < ngrp else 1
                    ot = out_pool.tile([128, GRP * C], fp32)
                    for k in range(nblk):
                        ps = psum_pool.tile([128, C], fp32)
                        first_block = g == 0 and k == 0
                        lhsT = w[:, 128:256] if first_block else w[:, 0:128]
                        nc.tensor.matmul(
                            ps[:, :],
                            lhsT,
                            rt[:, k * C : (k + 1) * C],
                            start=True,
                            stop=True,
                        )
                        # full-tile output copy (incl. carry rows, harmless)
                        cols = slice(k * C, (k + 1) * C)
                        if blk_idx % 2 == 0:
                            nc.scalar.copy(ot[:, cols], ps[:, :])
                        else:
                            nc.vector.tensor_copy(ot[:, cols], ps[:, :])
                        # propagate carries into the next block's rhs
                        if k + 1 < nblk:
                            cdst = rt[SPLIT : SPLIT + 2, (k + 1) * C : (k + 2) * C]
                        elif rhs_nxt[b] is not None:
                            cdst = rhs_nxt[b][SPLIT : SPLIT + 2, 0:C]
                        else:
                            cdst = None
                        if cdst is not None:
                            csrc = ps[SPLIT : SPLIT + 2, :]
                            if blk_idx % 2 == 0:
                                nc.vector.tensor_copy(cdst, csrc)
                            else:
                                nc.scalar.copy(cdst, csrc)
                        blk_idx += 1
                    # DMA the group's outputs to DRAM (ACT-side HWDGE ring)
                    t0 = g * GRP * T
                    if g < ngrp:
                        dst = y_ap[b, t0 : t0 + GRP * T, :].rearrange(
                            "(blk t) c -> t blk c", t=T
                        )
                        src = ot[:, :].rearrange("t (blk c) -> t blk c", blk=GRP)
                        nc.scalar.dma_start(dst[0:SPLIT], src[0:SPLIT])
                        nc.scalar.dma_start(dst[SPLIT:T], src[SPLIT + 2 : T + 2])
                    else:
                        nc.scalar.dma_start(
                            y_ap[b, t0 : t0 + TAIL, :], ot[0:TAIL, 0:C]
                        )
                rhs_cur = rhs_nxt
    nc.compile()
    return nc


def _amat_np():
    """Both lhsT matrices packed as one [128, 256] input."""
    out = np.zeros((128, 256), dtype=np.float32)
    out[:, 0:128] = _build_A().T
    out[:, 128:256] = _build_A0().T
    return out


_CACHED = {}


def _get_nc():
    if "nc" not in _CACHED:
        _CACHED["nc"] = build_bass()
    return _CACHED["nc"]


def kernel(**inputs: np.ndarray) -> np.ndarray:
    from concourse.bass_utils import run_bass_kernel_spmd

    x = np.ascontiguousarray(inputs["x"], dtype=np.float32)
    assert x.shape == (B_FULL, L, C), x.shape

    amat = _amat_np()

    nc = _get_nc()
    in_maps = [
        {"x": x[i * B_PER_CORE : (i + 1) * B_PER_CORE], "amat": amat}
        for i in range(N_CORES)
    ]
    res = run_bass_kernel_spmd(nc, in_maps, core_ids=list(range(N_CORES)))
    out = np.concatenate([r["y"] for r in res.results], axis=0)
    return out



# revision 3
# speedup vs baseline: 34.0008x; 34.0008x over previous
"""DEMA (double exponential moving average) Trainium2 Bass kernel.

Problem: x [32, 4096, 512] f32; y = 2*EMA(x) - EMA(EMA(x)) along time axis
(L=4096), alpha=0.1, with y_0 = x_0 initial condition.

Strategy
--------
Data-parallel over batch: 8 cores x 4 batch rows each (no communication).

DEMA is a linear map y = M x along time with an impulse response that decays
like (k+1)*0.9^k -- below 2e-6 beyond lag 128. So M is effectively banded:
with time blocks of T=128, out_blk_i = W1 @ x_blk_{i-1} + W0 @ x_blk_i
(two accumulating tensor-engine matmuls per block, no serial carry chain;
block 0 uses an exact first-block matrix Wf that encodes the y_0 = x_0
initial condition). Banded truncation error ~2e-6.

The problem is memory-bound (HBM-per-core ~358 GB/s), so all HBM traffic is
bf16: the host casts x f32 -> bf16 before upload and the kernel writes bf16
outputs which the host upcasts. This halves DMA bytes vs f32 and runs the
matmuls at 1 cycle/row. End-to-end rel err ~3e-3 (tolerance 2e-2).
"""

import numpy as np
import ml_dtypes

ALPHA = 0.1
BETA = 1.0 - ALPHA
B_FULL, L, C = 32, 4096, 512
N_CORES = 8
B_PER_CORE = B_FULL // N_CORES  # 4
T = 128  # time steps per block = partition dim
NBLK = L // T  # 32 blocks, no tail
GRP = 8  # blocks per DMA group (~1 MB bf16 transfers)
NGRP = NBLK // GRP  # 4 groups per batch row
BF16_NP = ml_dtypes.bfloat16


def _build_weights(dtype=np.float64):
    """Exact DEMA operator on 3T steps -> (Wf, W1, W0) block matrices."""
    n = 3 * T
    A = np.zeros((n, n), dtype)
    for t in range(1, n):
        s = np.arange(1, t + 1)
        A[t, s] = ALPHA * BETA ** (t - s)
        A[t, 0] = BETA**t
    A[0, 0] = 1.0
    M = 2 * A - A @ A
    Wf = M[0:T, 0:T]  # first block: exact, includes y_0 = x_0 init
    W1 = M[2 * T : 3 * T, T : 2 * T]  # steady state, prev-block columns
    W0 = M[2 * T : 3 * T, 2 * T : 3 * T]  # steady state, same-block columns
    return Wf, W1, W0


def _wmat_np():
    """lhsT matrices (transposed for the PE) packed as one [128, 384] bf16."""
    Wf, W1, W0 = _build_weights()
    out = np.zeros((T, 3 * T), dtype=BF16_NP)
    out[:, 0:T] = Wf.T.astype(BF16_NP)
    out[:, T : 2 * T] = W1.T.astype(BF16_NP)
    out[:, 2 * T : 3 * T] = W0.T.astype(BF16_NP)
    return out


def build_bass(l_mult=1):
    """Emit the per-core Bass/Tile program.

    l_mult > 1 builds a work-scaled timing variant (longer time axis) with
    identical per-block structure; only used by test.py."""
    import concourse.bass as bass
    import concourse.bacc as bacc
    import concourse.mybir as mybir
    from concourse import tile

    l_total = L * l_mult
    nblk = l_total // T
    ngrp = nblk // GRP

    bf16 = mybir.dt.bfloat16
    fp32 = mybir.dt.float32
    nc = bacc.Bacc(
        "TRN2", target_bir_lowering=False, debug=False, num_devices=N_CORES
    )

    x = nc.dram_tensor("x", [B_PER_CORE, l_total, C], bf16, kind="ExternalInput")
    wmat = nc.dram_tensor("wmat", [128, 3 * T], bf16, kind="ExternalInput")
    y = nc.dram_tensor("y", [B_PER_CORE, l_total, C], bf16, kind="ExternalOutput")
    x_ap, y_ap = x.ap(), y.ap()

    with tile.TileContext(nc) as tc:
        with (
            tc.tile_pool(name="w", bufs=1) as w_pool,
            tc.tile_pool(name="rhs", bufs=4) as rhs_pool,
            tc.tile_pool(name="out", bufs=4) as out_pool,
            tc.tile_pool(name="psum", bufs=8, space="PSUM") as psum_pool,
        ):
            w = w_pool.tile([128, 3 * T], bf16)
            nc.sync.dma_start(w[:, :], wmat.ap()[:, :])
            w_f = w[:, 0:T]
            w_1 = w[:, T : 2 * T]
            w_0 = w[:, 2 * T : 3 * T]

            def load_group(b, g):
                t0 = g * GRP * T
                rt = rhs_pool.tile([128, GRP * C], bf16)
                src = x_ap[b, t0 : t0 + GRP * T, :].rearrange(
                    "(blk t) c -> t blk c", t=T
                )
                dst = rt[:, :].rearrange("t (blk c) -> t blk c", blk=GRP)
                nc.sync.dma_start(dst, src)
                return rt

            blk_idx = 0
            for b in range(B_PER_CORE):
                rt_prev = None  # previous group's rhs tile (for W1 input)
                rt_cur = load_group(b, 0)
                for g in range(ngrp):
                    rt_nxt = load_group(b, g + 1) if g + 1 < ngrp else None
                    ot = out_pool.tile([128, GRP * C], bf16)
                    for k in range(GRP):
                        ps = psum_pool.tile([128, C], fp32)
                        cur = rt_cur[:, k * C : (k + 1) * C]
                        if g == 0 and k == 0:
                            nc.tensor.matmul(ps[:, :], w_f, cur, start=True, stop=True)
                        else:
                            prev = (
                                rt_cur[:, (k - 1) * C : k * C]
                                if k > 0
                                else rt_prev[:, (GRP - 1) * C : GRP * C]
                            )
                            nc.tensor.matmul(
                                ps[:, :], w_1, prev, start=True, stop=False
                            )
                            nc.tensor.matmul(
                                ps[:, :], w_0, cur, start=False, stop=True
                            )
                        cols = slice(k * C, (k + 1) * C)
                        if blk_idx % 2 == 0:
                            nc.scalar.copy(ot[:, cols], ps[:, :])
                        else:
                            nc.vector.tensor_copy(ot[:, cols], ps[:, :])
                        blk_idx += 1
                    t0 = g * GRP * T
                    dst = y_ap[b, t0 : t0 + GRP * T, :].rearrange(
                        "(blk t) c -> t blk c", t=T
                    )
                    src = ot[:, :].rearrange("t (blk c) -> t blk c", blk=GRP)
                    nc.scalar.dma_start(dst, src)
                    rt_prev, rt_cur = rt_cur, rt_nxt
    nc.compile()
    return nc


def make_in_maps(x_full, l_mult=1):
    """Per-core input dicts for run_bass_kernel_spmd (host casts to bf16)."""
    xb = np.ascontiguousarray(x_full).astype(BF16_NP)
    wmat = _wmat_np()
    return [
        {"x": xb[i * B_PER_CORE : (i + 1) * B_PER_CORE], "wmat": wmat}
        for i in range(N_CORES)
    ]


def make_big_maps(x_big):
    """Input dicts for the work-scaled timing variant (one core's x, any L)."""
    xb = np.ascontiguousarray(x_big).astype(BF16_NP)
    wmat = _wmat_np()
    return [{"x": xb, "wmat": wmat} for _ in range(N_CORES)]


_CACHED = {}


def _get_nc():
    if "nc" not in _CACHED:
        _CACHED["nc"] = build_bass()
    return _CACHED["nc"]


def kernel(**inputs: np.ndarray) -> np.ndarray:
    from concourse.bass_utils import run_bass_kernel_spmd

    x = np.ascontiguousarray(inputs["x"], dtype=np.float32)
    assert x.shape == (B_FULL, L, C), x.shape

    nc = _get_nc()
    in_maps = make_in_maps(x)
    res = run_bass_kernel_spmd(nc, in_maps, core_ids=list(range(N_CORES)))
    out = np.concatenate([r["y"] for r in res.results], axis=0)
    return out.astype(np.float32)
